# revision 30
# baseline (speedup 1.0000x reference)
"""Two-layer GAT on 8 Trainium2 NeuronCores.

Sharding: destination-node partitioning (1250 dst nodes per core, padded to
1280).  Each core computes the dense feature matmul for its own node chunk,
feature+logit tables are AllGathered in two halves, and each core processes
the edges whose destination lands in its chunk: indexed row gathers
(dma_gather, split across all 4 SWDGE queues so all Q7 cpu pairs generate
descriptors concurrently), edge softmax without max-subtraction, and
aggregation as one-hot scatter-matmuls on the tensor engine.

v2 structure: node rows use half-split global numbering so each AllGather
half is contiguous; each chunk's edges are split by source half (A = rows
0:640 of each core, B = rest) and each layer runs two passes (all chunks'
A-half edges first, then B-half), with PSUM partials parked in SBUF f32
accumulators between passes.  This lets edge processing start as soon as the
first AllGather half lands and hides the second half + inter-core skew.
Feature columns are head-interleaved (static W1 column permutation) so the
per-edge exp-scale multiply broadcasts along a packed last dim (DVE 2x),
attention projections are folded into W1/W2 on the host (no phase A), the
one-hot scatter tables are fp8 (exact 0/1, half the DMA), and the ELU
min/exp runs on the Scalar engine.
"""
import numpy as np
import ml_dtypes

N_NODES = 10000
N_EDGES = 320000
IN_DIM = 512
HID = 64
H0 = 8
OUT_D = 64
NEG = 0.2
NCORES = 8
NPER = 1250          # real nodes per core
LOC = 1280           # padded rows per core
HL = LOC // 2        # 640: rows per half per core
FULL = LOC * NCORES  # 10240 padded-global rows
HFULL = FULL // 2    # 5120
NCHUNK = 10          # dst chunks (of 128) per core
SLAB_BATCH = 4
FXW = 640            # u16 cols per fx1 row (512 feat bf16 + 8 el f32 + pad)

_cache = {}
_patched = {}


def _gather_fn():
    """dma_gather with the elem_size%256 assert relaxed (transpose-only
    restriction; the non-transpose Q7 path handles any packet size)."""
    if "fn" not in _patched:
        import inspect
        import textwrap
        import concourse.bass as cbass
        src = inspect.getsource(cbass.BassGpSimd.dma_gather)
        src = textwrap.dedent(src)
        src = src.replace(
            "elem_size_bytes > 0 and elem_size_bytes % 256 == 0",
            "elem_size_bytes > 0")
        ns = dict(vars(cbass))
        exec(src, ns)
        _patched["fn"] = ns["dma_gather"]
    return _patched["fn"]


def _build(KA, KB):
    import concourse.bacc as bacc
    import concourse.mybir as mybir
    import concourse.tile as tile

    f32 = mybir.dt.float32
    bf16 = mybir.dt.bfloat16
    fp8 = mybir.dt.float8e4
    u16 = mybir.dt.uint16
    i16 = mybir.dt.int16
    Alu = mybir.AluOpType
    Act = mybir.ActivationFunctionType

    SA = sum(KA)
    SB = sum(KB)
    offA = np.concatenate([[0], np.cumsum(KA)]).astype(int)
    offB = np.concatenate([[0], np.cumsum(KB)]).astype(int)
    KT = max(max(KA), max(KB))
    K1 = IN_DIM // 128  # 4

    nc = bacc.Bacc("TRN2", target_bir_lowering=False, debug=False,
                   enable_asserts=True, num_devices=NCORES,
                   num_swdge_queues=4)

    # ---------------- I/O ----------------
    hT_d = nc.dram_tensor("hT", [IN_DIM, LOC], bf16, kind="ExternalInput")
    W1F_d = nc.dram_tensor("W1F", [IN_DIM, 528], bf16, kind="ExternalInput")
    W2F_d = nc.dram_tensor("W2F", [IN_DIM, 66], bf16, kind="ExternalInput")
    srcIA_d = nc.dram_tensor("srcIA", [128, SA * 8], i16, kind="ExternalInput")
    srcIB_d = nc.dram_tensor("srcIB", [128, SB * 8], i16, kind="ExternalInput")
    dofA_d = nc.dram_tensor("dofA", [128, SA * 128], fp8, kind="ExternalInput")
    dofB_d = nc.dram_tensor("dofB", [128, SB * 128], fp8, kind="ExternalInput")
    stA_d = nc.dram_tensor("stA", [128, SA * 128], fp8, kind="ExternalInput")
    stB_d = nc.dram_tensor("stB", [128, SB * 128], fp8, kind="ExternalInput")
    id_d = nc.dram_tensor("ident", [128, 128], f32, kind="ExternalInput")
    y_d = nc.dram_tensor("y", [LOC, OUT_D], f32, kind="ExternalOutput")

    # ---------------- internal DRAM ----------------
    fx1_lA = nc.dram_tensor("fx1_lA", [HL, FXW], u16)
    fx1_lB = nc.dram_tensor("fx1_lB", [HL, FXW], u16)
    fx1_full = nc.dram_tensor("fx1_full", [FULL, FXW], u16, addr_space="Shared")
    fx2_lA = nc.dram_tensor("fx2_lA", [HL, 128], u16)
    fx2_lB = nc.dram_tensor("fx2_lB", [HL, 128], u16)
    fx2_full = nc.dram_tensor("fx2_full", [FULL, 128], u16, addr_space="Shared")

    RG = [list(range(NCORES))]

    with tile.TileContext(nc) as tc:
        with (
            tc.tile_pool(name="const", bufs=1) as cp,
            tc.tile_pool(name="work", bufs=2) as wp,
        ):
            def load_const(name, dram, shape, dtype):
                t = cp.tile(shape, dtype, tag=name)
                nc.sync.dma_start(t[:], dram)
                return t

            # hT + W1F load first: phase B's first matmul needs hT strip 0
            hw_pool = tc.tile_pool(name="hw", bufs=1)
            hp = hw_pool.__enter__()
            hT_b = hp.tile([128, K1, LOC], bf16, tag="hT_b")
            for s in range(K1):
                nc.sync.dma_start(hT_b[:, s, :],
                                  hT_d[s * 128:(s + 1) * 128, :])
            hT_t = [hT_b[:, s, :] for s in range(K1)]
            W1F_b = load_const("W1F_b", W1F_d[:].rearrange("(s p) f -> p s f", p=128),
                               [128, K1, 528], bf16)
            id_t = load_const("ident", id_d[:], [128, 128], f32)
            srcIA_t = load_const("srcIA", srcIA_d[:], [128, SA * 8], i16)
            srcIB_t = load_const("srcIB", srcIB_d[:], [128, SB * 8], i16)
            W2F_b = load_const("W2F_b", W2F_d[:].rearrange("(s p) f -> p s f", p=128),
                               [128, K1, 66], bf16)
            # preload pass-A one-hot tables during the (DMA-idle) preamble;
            # they are shared by L1-A and L2-A
            dofA_t = load_const("dofA_t", dofA_d[:], [128, SA * 128], fp8)
            stA_t = load_const("stA_t", stA_d[:], [128, SA * 128], fp8)
            preA = (dofA_t, stA_t)
            er1_sb, er2_sb = [], []
            acc1_sb, accS1_sb = [], []
            acc2_sb, accS2_sb = [], []

            # ---- phase B: layer-1 feature + logit tables, split AllGather ----
            with tc.tile_pool(name="ppB", bufs=2, space="PSUM") as ppB:
                for nb in range(NCHUNK):
                    ps = ppB.tile([128, 512], f32, tag="feat")
                    psLR = ppB.tile([128, 16], f32, tag="featlr")
                    blk = slice(nb * 128, (nb + 1) * 128)
                    for s in range(K1):
                        st_ = (s == 0)
                        sp_ = (s == K1 - 1)
                        nc.tensor.matmul(ps[:], hT_t[s][:, blk],
                                         W1F_b[:, s, 0:512], start=st_, stop=sp_)
                        nc.tensor.matmul(psLR[:], hT_t[s][:, blk],
                                         W1F_b[:, s, 512:528], start=st_, stop=sp_)
                    fx = wp.tile([128, FXW], u16, tag="fx")
                    nc.vector.tensor_copy(fx[:, 0:512].bitcast(bf16), ps[:])
                    nc.vector.tensor_copy(fx[:, 512:528].bitcast(f32),
                                          psLR[:, 0:8])
                    if nb < 5:
                        nc.sync.dma_start(fx1_lA[nb * 128:(nb + 1) * 128, :], fx[:])
                    else:
                        nc.sync.dma_start(fx1_lB[(nb - 5) * 128:(nb - 4) * 128, :],
                                          fx[:])
                    er = cp.tile([128, 8], bf16, tag=f"er1c{nb}", name=f"er1c{nb}")
                    nc.vector.tensor_copy(er[:], psLR[:, 8:16])
                    er1_sb.append(er)
                    if nb == 4:
                        nc.gpsimd.collective_compute(
                            "AllGather", mybir.AluOpType.bypass,
                            replica_groups=RG,
                            ins=[fx1_lA[:]], outs=[fx1_full[0:HFULL, :]])
                nc.gpsimd.collective_compute(
                    "AllGather", mybir.AluOpType.bypass,
                    replica_groups=RG,
                    ins=[fx1_lB[:]], outs=[fx1_full[HFULL:FULL, :]])
            hw_pool.__exit__(None, None, None)

            # ---- layer-1 edge processing: pass A then pass B ----
            # pools are shared across the two passes so no pool-boundary
            # barrier blocks pass B's table prefetch during pass A / the AG
            def l1_pass(half, ks, offs, srcI_t, dof_d, st_d, tab,
                        ppC, ppT, ppD, ep, pre=None):
                    for ch in range(NCHUNK):
                        k = ks[ch]
                        o = offs[ch]
                        if pre is None:
                            sdt = ep.tile([128, KT * 128], fp8, tag="sdt")
                            nc.sync.dma_start(sdt[:, 0:k * 128],
                                              dof_d[:, o * 128:(o + k) * 128])
                            stt = ep.tile([128, KT * 128], fp8, tag="stt")
                            nc.sync.dma_start(stt[:, 0:k * 128],
                                              st_d[:, o * 128:(o + k) * 128])
                            so = 0
                        else:
                            sdt, stt = pre
                            so = o
                        erp = ppC.tile([128, KT * 8], f32, tag="erp", bufs=1)
                        for jj in range(k):
                            nc.tensor.matmul(erp[:, jj * 8:(jj + 1) * 8],
                                             sdt[:, (so + jj) * 128:
                                                 (so + jj + 1) * 128],
                                             er1_sb[ch][:],
                                             start=(jj == 0), stop=(jj == k - 1),
                                             skip_group_check=True)
                        pa = ppC.tile([128, 512], f32, tag="agg")
                        pss = ppC.tile([128, 8], f32, tag="ss")
                        g = ep.tile([128, KT, 528], u16, tag="g", bufs=5)
                        for q in range(4):
                            a = k * q // 4
                            b = k * (q + 1) // 4
                            if b <= a:
                                continue
                            _gather_fn()(
                                nc.gpsimd,
                                g[:, a:b, :], tab[:, 0:528],
                                srcI_t[:, (o + a) * 8:(o + b) * 8],
                                num_idxs=(b - a) * 128,
                                num_idxs_reg=(b - a) * 128,
                                elem_size=528, elem_step=FXW,
                                single_packet=True, queue_num=q)
                        for j0 in range(0, k, SLAB_BATCH):
                            nb_ = min(SLAB_BATCH, k - j0)
                            jsl = slice(j0, j0 + nb_)
                            lr = ep.tile([128, SLAB_BATCH, 8], f32, tag="lr",
                                         bufs=3)
                            nc.vector.tensor_tensor(
                                lr[:, 0:nb_, :],
                                g[:, jsl, 512:528].bitcast(f32),
                                erp[:, j0 * 8:(j0 + nb_) * 8]
                                    .rearrange("p (b n) -> p b n", n=8),
                                Alu.add)
                            nc.vector.scalar_tensor_tensor(
                                lr[:, 0:nb_, :], lr[:, 0:nb_, :], NEG,
                                lr[:, 0:nb_, :], Alu.mult, Alu.max)
                            exb = ep.tile([128, SLAB_BATCH, 8], bf16, tag="exb",
                                          bufs=3)
                            nc.scalar.activation(exb[:, 0:nb_, :],
                                                 lr[:, 0:nb_, :], Act.Exp)
                            xt = ep.tile([128, SLAB_BATCH, 512], bf16, tag="xt",
                                         bufs=4)
                            nc.vector.tensor_tensor(
                                xt[:, 0:nb_, :]
                                    .rearrange("p b (d h) -> p b d h", h=H0),
                                g[:, jsl, 0:512].bitcast(bf16)
                                    .rearrange("p b (d h) -> p b d h", h=H0),
                                exb[:, 0:nb_, :].unsqueeze(2)
                                    .broadcast_to([128, nb_, HID, H0]),
                                Alu.mult)
                            for j in range(nb_):
                                jj = j0 + j
                                stj = stt[:, (so + jj) * 128:
                                           (so + jj + 1) * 128]
                                nc.tensor.matmul(pa[:], stj, xt[:, j, :],
                                                 start=(jj == 0),
                                                 stop=(jj == k - 1))
                                nc.tensor.matmul(pss[:], stj, exb[:, j, :],
                                                 start=(jj == 0),
                                                 stop=(jj == k - 1))
                        if half == "A":
                            a1 = cp.tile([128, 512], f32, tag=f"a1_{ch}",
                                         name=f"a1_{ch}")
                            nc.vector.tensor_copy(a1[:], pa[:])
                            s1 = cp.tile([128, 8], f32, tag=f"s1_{ch}",
                                         name=f"s1_{ch}")
                            nc.vector.tensor_copy(s1[:], pss[:])
                            acc1_sb.append(a1)
                            accS1_sb.append(s1)
                            continue
                        # ---- pass B tail: combine + normalize + ELU + layer-2
                        # feature row ----
                        sden = ep.tile([128, 8], f32, tag="sden")
                        nc.vector.tensor_tensor(sden[:], pss[:],
                                                accS1_sb[ch][:], Alu.add)
                        nc.vector.tensor_scalar(sden[:], sden[:], 1e-30, None,
                                                Alu.max)
                        rec = ep.tile([128, 8], f32, tag="rec")
                        nc.vector.reciprocal(rec[:], sden[:])
                        xs = ep.tile([128, 512], f32, tag="xs", bufs=1)
                        nc.vector.tensor_tensor(xs[:], pa[:], acc1_sb[ch][:],
                                                Alu.add)
                        x1 = ep.tile([128, 512], f32, tag="x1", bufs=1)
                        nc.vector.tensor_tensor(
                            x1[:].rearrange("p (d h) -> p d h", h=H0),
                            xs[:].rearrange("p (d h) -> p d h", h=H0),
                            rec[:].unsqueeze(1).broadcast_to([128, HID, H0]),
                            Alu.mult)
                        # ELU: relu(v) - 1 + exp(min(v, 0)); min via relu(-v)
                        tng = ep.tile([128, 512], f32, tag="tng", bufs=1)
                        nc.scalar.activation(tng[:], x1[:], Act.Relu, scale=-1.0)
                        texp = ep.tile([128, 512], f32, tag="texp", bufs=1)
                        nc.scalar.activation(texp[:], tng[:], Act.Exp, scale=-1.0)
                        trelu = ep.tile([128, 512], f32, tag="trelu", bufs=1)
                        nc.scalar.activation(trelu[:], x1[:], Act.Relu)
                        xe = ep.tile([128, 512], f32, tag="xe", bufs=1)
                        nc.vector.scalar_tensor_tensor(xe[:], texp[:], -1.0,
                                                       trelu[:], Alu.add, Alu.add)
                        xTc = []
                        for s in range(K1):
                            tp = ppT.tile([128, 128], f32, tag="tp", bufs=1)
                            nc.tensor.transpose(tp[:],
                                                xe[:, s * 128:(s + 1) * 128],
                                                id_t[:])
                            xc = wp.tile([128, 128], bf16, tag=f"xTc{s}")
                            nc.vector.tensor_copy(xc[:], tp[:])
                            xTc.append(xc)
                        psD = ppD.tile([128, 66], f32, tag="feat2", bufs=2)
                        for s in range(K1):
                            nc.tensor.matmul(psD[:], xTc[s][:], W2F_b[:, s, :],
                                             start=(s == 0), stop=(s == K1 - 1))
                        fx2 = wp.tile([128, 128], u16, tag="fx2")
                        nc.vector.tensor_copy(fx2[:, 0:64].bitcast(bf16),
                                              psD[:, 0:64])
                        nc.vector.tensor_copy(fx2[:, 64:66].bitcast(f32),
                                              psD[:, 64:65])
                        er2 = cp.tile([128, 1], bf16, tag=f"er2c{ch}",
                                      name=f"er2c{ch}")
                        nc.vector.tensor_copy(er2[:], psD[:, 65:66])
                        er2_sb.append(er2)
                        if ch < 5:
                            nc.sync.dma_start(
                                fx2_lA[ch * 128:(ch + 1) * 128, :], fx2[:])
                        else:
                            nc.sync.dma_start(
                                fx2_lB[(ch - 5) * 128:(ch - 4) * 128, :], fx2[:])
                        if ch == 4:
                            nc.gpsimd.collective_compute(
                                "AllGather", mybir.AluOpType.bypass,
                                replica_groups=RG,
                                ins=[fx2_lA[:]], outs=[fx2_full[0:HFULL, :]])

            with (
                tc.tile_pool(name="ppC", bufs=2, space="PSUM") as ppC,
                tc.tile_pool(name="ppT", bufs=2, space="PSUM") as ppT,
                tc.tile_pool(name="ppD", bufs=2, space="PSUM") as ppD,
                tc.tile_pool(name="edge", bufs=2) as ep,
            ):
                l1_pass("A", KA, offA, srcIA_t, dofA_d[:], stA_d[:],
                        fx1_full[0:HFULL, :], ppC, ppT, ppD, ep, pre=preA)
                l1_pass("B", KB, offB, srcIB_t, dofB_d[:], stB_d[:],
                        fx1_full[HFULL:FULL, :], ppC, ppT, ppD, ep)

            nc.gpsimd.collective_compute(
                "AllGather", mybir.AluOpType.bypass,
                replica_groups=RG,
                ins=[fx2_lB[:]], outs=[fx2_full[HFULL:FULL, :]])

            # ---- layer-2 edge processing: pass A then pass B ----
            def l2_pass(half, ks, offs, srcI_t, dof_d, st_d, tab, ppE, e2,
                        pre=None):
                    for ch in range(NCHUNK):
                        k = ks[ch]
                        o = offs[ch]
                        if pre is None:
                            sdt = e2.tile([128, KT * 128], fp8, tag="sdt2")
                            nc.sync.dma_start(sdt[:, 0:k * 128],
                                              dof_d[:, o * 128:(o + k) * 128])
                            stt = e2.tile([128, KT * 128], fp8, tag="stt2")
                            nc.sync.dma_start(stt[:, 0:k * 128],
                                              st_d[:, o * 128:(o + k) * 128])
                            so = 0
                        else:
                            sdt, stt = pre
                            so = o
                        erp2 = ppE.tile([128, KT], f32, tag="erp2", bufs=1)
                        for jj in range(k):
                            nc.tensor.matmul(erp2[:, jj:jj + 1],
                                             sdt[:, (so + jj) * 128:
                                                 (so + jj + 1) * 128],
                                             er2_sb[ch][:],
                                             start=(jj == 0), stop=(jj == k - 1),
                                             skip_group_check=True)
                        pa2 = ppE.tile([128, 64], f32, tag="agg2")
                        pss2 = ppE.tile([128, 1], f32, tag="ss2")
                        g2 = e2.tile([128, KT, 128], u16, tag="g2", bufs=6)
                        for q in range(4):
                            a = k * q // 4
                            b = k * (q + 1) // 4
                            if b <= a:
                                continue
                            nc.gpsimd.dma_gather(
                                g2[:, a:b, :], tab,
                                srcI_t[:, (o + a) * 8:(o + b) * 8],
                                num_idxs=(b - a) * 128,
                                num_idxs_reg=(b - a) * 128,
                                elem_size=128, single_packet=True,
                                queue_num=q)
                        for j0 in range(0, k, SLAB_BATCH):
                            nb_ = min(SLAB_BATCH, k - j0)
                            jsl = slice(j0, j0 + nb_)
                            lr2 = e2.tile([128, SLAB_BATCH, 1], f32, tag="lr2",
                                          bufs=3)
                            nc.vector.tensor_tensor(
                                lr2[:, 0:nb_, :],
                                g2[:, jsl, 64:66].bitcast(f32),
                                erp2[:, j0:j0 + nb_]
                                    .rearrange("p (b n) -> p b n", n=1),
                                Alu.add)
                            nc.vector.scalar_tensor_tensor(
                                lr2[:, 0:nb_, :], lr2[:, 0:nb_, :], NEG,
                                lr2[:, 0:nb_, :], Alu.mult, Alu.max)
                            exb2 = e2.tile([128, SLAB_BATCH, 1], bf16,
                                           tag="exb2", bufs=3)
                            nc.scalar.activation(exb2[:, 0:nb_, :],
                                                 lr2[:, 0:nb_, :], Act.Exp)
                            xt2 = e2.tile([128, SLAB_BATCH, 64], bf16,
                                          tag="xt2", bufs=3)
                            nc.vector.tensor_tensor(
                                xt2[:, 0:nb_, :],
                                g2[:, jsl, 0:64].bitcast(bf16),
                                exb2[:, 0:nb_, :]
                                    .broadcast_to([128, nb_, 64]),
                                Alu.mult)
                            for j in range(nb_):
                                jj = j0 + j
                                stj = stt[:, (so + jj) * 128:
                                           (so + jj + 1) * 128]
                                nc.tensor.matmul(pa2[:], stj, xt2[:, j, :],
                                                 start=(jj == 0),
                                                 stop=(jj == k - 1))
                                nc.tensor.matmul(pss2[:], stj, exb2[:, j, :],
                                                 start=(jj == 0),
                                                 stop=(jj == k - 1))
                        if half == "A":
                            a2 = cp.tile([128, 64], f32, tag=f"a2_{ch}",
                                         name=f"a2_{ch}")
                            nc.vector.tensor_copy(a2[:], pa2[:])
                            s2 = cp.tile([128, 1], f32, tag=f"s2_{ch}",
                                         name=f"s2_{ch}")
                            nc.vector.tensor_copy(s2[:], pss2[:])
                            acc2_sb.append(a2)
                            accS2_sb.append(s2)
                            continue
                        sden2 = e2.tile([128, 1], f32, tag="sden2")
                        nc.vector.tensor_tensor(sden2[:], pss2[:],
                                                accS2_sb[ch][:], Alu.add)
                        nc.vector.tensor_scalar(sden2[:], sden2[:], 1e-30,
                                                None, Alu.max)
                        rec2 = e2.tile([128, 1], f32, tag="rec2")
                        nc.vector.reciprocal(rec2[:], sden2[:])
                        xo = e2.tile([128, 64], f32, tag="xo")
                        nc.vector.tensor_tensor(xo[:], pa2[:], acc2_sb[ch][:],
                                                Alu.add)
                        outf = e2.tile([128, 64], f32, tag="outf")
                        nc.vector.tensor_tensor(
                            outf[:], xo[:],
                            rec2[:].broadcast_to([128, 64]), Alu.mult)
                        nc.sync.dma_start(y_d[ch * 128:(ch + 1) * 128, :],
                                          outf[:])

            with (
                tc.tile_pool(name="ppE", bufs=2, space="PSUM") as ppE,
                tc.tile_pool(name="edge2", bufs=2) as e2,
            ):
                l2_pass("A", KA, offA, srcIA_t, dofA_d[:], stA_d[:],
                        fx2_full[0:HFULL, :], ppE, e2, pre=preA)
                l2_pass("B", KB, offB, srcIB_t, dofB_d[:], stB_d[:],
                        fx2_full[HFULL:FULL, :], ppE, e2)

    nc.compile()
    return nc


def _wrap_idx(a):
    """flat int array -> [128, n//16] int16 dma_gather index layout."""
    w = a.reshape(-1, 16).T.astype(np.int16)
    return np.tile(w, (8, 1))


def _prep_inputs(h, src, dst, W1, attn_l1, attn_r1, W2, attn_l2, attn_r2):
    src = np.asarray(src)
    dst = np.asarray(dst)
    h = np.asarray(h, dtype=np.float32)
    W1 = np.asarray(W1, dtype=np.float32)
    W2 = np.asarray(W2, dtype=np.float32)
    al1 = np.asarray(attn_l1, np.float32)
    ar1 = np.asarray(attn_r1, np.float32)
    al2 = np.asarray(attn_l2, np.float32).reshape(-1)
    ar2 = np.asarray(attn_r2, np.float32).reshape(-1)

    # head-interleaved feature column permutation: new col d*8+h <- old h*64+d
    jj = np.arange(IN_DIM)
    pw1 = (jj % H0) * HID + (jj // H0)     # old col for new col j
    W1X = W1[:, pw1]
    Al1f = np.zeros((IN_DIM, H0), np.float32)
    Ar1f = np.zeros((IN_DIM, H0), np.float32)
    Al1f[jj, jj % H0] = al1[jj % H0, jj // H0]
    Ar1f[jj, jj % H0] = ar1[jj % H0, jj // H0]
    W1F = np.concatenate([W1X, W1X @ Al1f, W1X @ Ar1f], axis=1)
    # rows of W2 permuted to match interleaved layer-1 output columns
    W2X = W2[pw1, :]
    W2F = np.concatenate([W2X, (W2X @ al2)[:, None], (W2X @ ar2)[:, None]],
                         axis=1)

    # half-split global row numbering
    nn = np.arange(N_NODES)
    cc = nn // NPER
    ll = nn % NPER
    gid = np.where(ll < HL, cc * HL + ll, HFULL + cc * HL + (ll - HL))

    core_of = dst // NPER
    dloc_all = dst % NPER

    edA, edB = [], []          # per core: lists of (src_gid, dloc) per chunk
    nAs = np.zeros((NCORES, NCHUNK), int)
    nBs = np.zeros((NCORES, NCHUNK), int)
    for c in range(NCORES):
        ids = np.nonzero(core_of == c)[0]
        dl = dloc_all[ids]
        ch = dl // 128
        gsrc = gid[src[ids]]
        isA = gsrc < HFULL
        rowsA, rowsB = [], []
        for k in range(NCHUNK):
            mA = (ch == k) & isA
            mB = (ch == k) & ~isA
            iA = ids[mA][np.argsort(gsrc[mA], kind="stable")]
            iB = ids[mB][np.argsort(gsrc[mB], kind="stable")]
            rowsA.append((gid[src[iA]], dloc_all[iA] - 128 * k))
            rowsB.append((gid[src[iB]] - HFULL, dloc_all[iB] - 128 * k))
            nAs[c, k] = len(iA)
            nBs[c, k] = len(iB)
        edA.append(rowsA)
        edB.append(rowsB)

    KA = tuple(int(np.ceil(nAs[:, k].max() / 128)) for k in range(NCHUNK))
    KB = tuple(int(np.ceil(nBs[:, k].max() / 128)) for k in range(NCHUNK))

    def build_tables(rows_per_core_chunk, KS):
        """-> (srcI wrapped, dof one-hot fp8, st one-hot fp8) per core."""
        S = sum(KS)
        rng128 = np.arange(128)
        out = []
        for c in range(NCORES):
            src_g = np.zeros(S * 128, np.int64)
            doff = np.full(S * 128, 200, np.int64)
            pos = 0
            for k in range(NCHUNK):
                gs, dl = rows_per_core_chunk[c][k]
                n = len(gs)
                src_g[pos:pos + n] = gs
                doff[pos:pos + n] = dl
                pos += KS[k] * 128
            doh = (doff[None, :] == rng128[:, None]).astype(
                ml_dtypes.float8_e4m3)
            D = doff.reshape(-1, 128)
            soh = np.ascontiguousarray(
                (D[:, :, None] == rng128[None, None, :])
                .transpose(1, 0, 2).reshape(128, S * 128)).astype(
                    ml_dtypes.float8_e4m3)
            out.append((_wrap_idx(src_g), doh, soh))
        return out

    tabA = build_tables(edA, KA)
    tabB = build_tables(edB, KB)

    ident = np.eye(128, dtype=np.float32)
    in_maps = []
    for c in range(NCORES):
        hc = np.zeros((IN_DIM, LOC), ml_dtypes.bfloat16)
        hc[:, :NPER] = h[c * NPER:(c + 1) * NPER].T.astype(ml_dtypes.bfloat16)
        in_maps.append({
            "hT": hc,
            "W1F": W1F.astype(ml_dtypes.bfloat16),
            "W2F": W2F.astype(ml_dtypes.bfloat16),
            "srcIA": tabA[c][0], "srcIB": tabB[c][0],
            "dofA": tabA[c][1], "dofB": tabB[c][1],
            "stA": tabA[c][2], "stB": tabB[c][2],
            "ident": ident,
        })
    return KA, KB, in_maps


def kernel(h, src, dst, W1, attn_l1, attn_r1, W2, attn_l2, attn_r2,
           _trace=False):
    from concourse.bass_utils import run_bass_kernel_spmd

    KA, KB, in_maps = _prep_inputs(h, src, dst, W1, attn_l1, attn_r1,
                                   W2, attn_l2, attn_r2)
    key = (KA, KB)
    if key not in _cache:
        _cache[key] = _build(KA, KB)
    nc = _cache[key]

    if _trace:
        _install_trace_hook()
    res = run_bass_kernel_spmd(nc, in_maps, list(range(NCORES)), trace=_trace)
    out = np.concatenate([res.results[c]["y"][:NPER] for c in range(NCORES)],
                         axis=0)
    if _trace:
        return out, res
    return out


def _install_trace_hook():
    import sys, types
    if "antenv.axon_hooks" in sys.modules:
        return
    try:
        import antenv
        from trn_agent_boot.trn_boot import _ntff_profile_via_ctypes
    except ImportError:
        return
    mod = types.ModuleType("antenv.axon_hooks")
    state = {"hook": None}
    mod.set_axon_ntff_profile_hook = lambda hk: state.__setitem__("hook", hk)
    mod.get_axon_ntff_profile_hook = lambda: state["hook"]
    sys.modules["antenv.axon_hooks"] = mod
    antenv.axon_hooks = mod
    try:
        mod.set_axon_ntff_profile_hook(
            _ntff_profile_via_ctypes("/opt/axon/libaxon_pjrt.so"))
    except Exception:
        pass


# revision 31
# speedup vs baseline: 1.0414x; 1.0414x over previous
"""Two-layer GAT on 8 Trainium2 NeuronCores.

Sharding: destination-node partitioning (1250 dst nodes per core, padded to
1280).  Each core computes the dense feature matmul for its own node chunk,
feature+logit tables are AllGathered in two halves, and each core processes
the edges whose destination lands in its chunk: indexed row gathers
(dma_gather, split across all 4 SWDGE queues so all Q7 cpu pairs generate
descriptors concurrently), edge softmax without max-subtraction, and
aggregation as one-hot scatter-matmuls on the tensor engine.

v2 structure: node rows use half-split global numbering so each AllGather
half is contiguous; each chunk's edges are split by source half (A = rows
0:640 of each core, B = rest) and each layer runs two passes (all chunks'
A-half edges first, then B-half), with PSUM partials parked in SBUF f32
accumulators between passes.  This lets edge processing start as soon as the
first AllGather half lands and hides the second half + inter-core skew.
Feature columns are head-interleaved (static W1 column permutation) so the
per-edge exp-scale multiply broadcasts along a packed last dim (DVE 2x),
attention projections are folded into W1/W2 on the host (no phase A), the
one-hot scatter tables are fp8 (exact 0/1, half the DMA), and the ELU
min/exp runs on the Scalar engine.
"""
import numpy as np
import ml_dtypes

N_NODES = 10000
N_EDGES = 320000
IN_DIM = 512
HID = 64
H0 = 8
OUT_D = 64
NEG = 0.2
NCORES = 8
NPER = 1250          # real nodes per core
LOC = 1280           # padded rows per core
HL = LOC // 2        # 640: rows per half per core
FULL = LOC * NCORES  # 10240 padded-global rows
HFULL = FULL // 2    # 5120
NCHUNK = 10          # dst chunks (of 128) per core
SLAB_BATCH = 4
FXW = 640            # u16 cols per fx1 row (512 feat bf16 + 8 el f32 + pad)

_cache = {}
_patched = {}


def _gather_fn():
    """dma_gather with the elem_size%256 assert relaxed (transpose-only
    restriction; the non-transpose Q7 path handles any packet size)."""
    if "fn" not in _patched:
        import inspect
        import textwrap
        import concourse.bass as cbass
        src = inspect.getsource(cbass.BassGpSimd.dma_gather)
        src = textwrap.dedent(src)
        src = src.replace(
            "elem_size_bytes > 0 and elem_size_bytes % 256 == 0",
            "elem_size_bytes > 0")
        ns = dict(vars(cbass))
        exec(src, ns)
        _patched["fn"] = ns["dma_gather"]
    return _patched["fn"]


def _build(KA, KB):
    import concourse.bacc as bacc
    import concourse.mybir as mybir
    import concourse.tile as tile

    f32 = mybir.dt.float32
    bf16 = mybir.dt.bfloat16
    fp8 = mybir.dt.float8e4
    u16 = mybir.dt.uint16
    i16 = mybir.dt.int16
    Alu = mybir.AluOpType
    Act = mybir.ActivationFunctionType

    SA = sum(KA)
    SB = sum(KB)
    offA = np.concatenate([[0], np.cumsum(KA)]).astype(int)
    offB = np.concatenate([[0], np.cumsum(KB)]).astype(int)
    KT = max(max(KA), max(KB))
    K1 = IN_DIM // 128  # 4

    nc = bacc.Bacc("TRN2", target_bir_lowering=False, debug=False,
                   enable_asserts=True, num_devices=NCORES,
                   num_swdge_queues=4)

    # ---------------- I/O ----------------
    hT_d = nc.dram_tensor("hT", [IN_DIM, LOC], bf16, kind="ExternalInput")
    W1F_d = nc.dram_tensor("W1F", [IN_DIM, 528], bf16, kind="ExternalInput")
    W2F_d = nc.dram_tensor("W2F", [IN_DIM, 66], bf16, kind="ExternalInput")
    srcIA_d = nc.dram_tensor("srcIA", [128, SA * 8], i16, kind="ExternalInput")
    srcIB_d = nc.dram_tensor("srcIB", [128, SB * 8], i16, kind="ExternalInput")
    dofA_d = nc.dram_tensor("dofA", [128, SA * 128], fp8, kind="ExternalInput")
    dofB_d = nc.dram_tensor("dofB", [128, SB * 128], fp8, kind="ExternalInput")
    stA_d = nc.dram_tensor("stA", [128, SA * 128], fp8, kind="ExternalInput")
    stB_d = nc.dram_tensor("stB", [128, SB * 128], fp8, kind="ExternalInput")
    id_d = nc.dram_tensor("ident", [128, 128], f32, kind="ExternalInput")
    y_d = nc.dram_tensor("y", [LOC, OUT_D], f32, kind="ExternalOutput")

    # ---------------- internal DRAM ----------------
    fx1_lA = nc.dram_tensor("fx1_lA", [HL, FXW], u16)
    fx1_lB = nc.dram_tensor("fx1_lB", [HL, FXW], u16)
    fx1_full = nc.dram_tensor("fx1_full", [FULL, FXW], u16, addr_space="Shared")
    fx2_lA = nc.dram_tensor("fx2_lA", [HL, 128], u16)
    fx2_lB = nc.dram_tensor("fx2_lB", [HL, 128], u16)
    fx2_full = nc.dram_tensor("fx2_full", [FULL, 128], u16, addr_space="Shared")

    RG = [list(range(NCORES))]

    with tile.TileContext(nc) as tc:
        with (
            tc.tile_pool(name="const", bufs=1) as cp,
            tc.tile_pool(name="work", bufs=2) as wp,
        ):
            def load_const(name, dram, shape, dtype):
                t = cp.tile(shape, dtype, tag=name)
                nc.sync.dma_start(t[:], dram)
                return t

            # hT + W1F load first: phase B's first matmul needs hT strip 0
            hw_pool = tc.tile_pool(name="hw", bufs=1)
            hp = hw_pool.__enter__()
            hT_b = hp.tile([128, K1, LOC], bf16, tag="hT_b")
            for s in range(K1):
                nc.sync.dma_start(hT_b[:, s, :],
                                  hT_d[s * 128:(s + 1) * 128, :])
            hT_t = [hT_b[:, s, :] for s in range(K1)]
            W1F_b = load_const("W1F_b", W1F_d[:].rearrange("(s p) f -> p s f", p=128),
                               [128, K1, 528], bf16)
            id_t = load_const("ident", id_d[:], [128, 128], f32)
            srcIA_t = load_const("srcIA", srcIA_d[:], [128, SA * 8], i16)
            srcIB_t = load_const("srcIB", srcIB_d[:], [128, SB * 8], i16)
            W2F_b = load_const("W2F_b", W2F_d[:].rearrange("(s p) f -> p s f", p=128),
                               [128, K1, 66], bf16)
            # preload pass-A one-hot tables during the (DMA-idle) preamble;
            # they are shared by L1-A and L2-A
            dofA_t = load_const("dofA_t", dofA_d[:], [128, SA * 128], fp8)
            stA_t = load_const("stA_t", stA_d[:], [128, SA * 128], fp8)
            preA = (dofA_t, stA_t)
            er1_sb, er2_sb = [], []
            acc1_sb, accS1_sb = [], []
            acc2_sb, accS2_sb = [], []

            # ---- phase B: layer-1 feature + logit tables, split AllGather ----
            with tc.tile_pool(name="ppB", bufs=2, space="PSUM") as ppB:
                for nb in range(NCHUNK):
                    ps = ppB.tile([128, 512], f32, tag="feat")
                    psLR = ppB.tile([128, 16], f32, tag="featlr")
                    blk = slice(nb * 128, (nb + 1) * 128)
                    for s in range(K1):
                        st_ = (s == 0)
                        sp_ = (s == K1 - 1)
                        nc.tensor.matmul(ps[:], hT_t[s][:, blk],
                                         W1F_b[:, s, 0:512], start=st_, stop=sp_)
                        nc.tensor.matmul(psLR[:], hT_t[s][:, blk],
                                         W1F_b[:, s, 512:528], start=st_, stop=sp_)
                    fx = wp.tile([128, FXW], u16, tag="fx")
                    nc.vector.tensor_copy(fx[:, 0:512].bitcast(bf16), ps[:])
                    nc.vector.tensor_copy(fx[:, 512:528].bitcast(f32),
                                          psLR[:, 0:8])
                    if nb < 5:
                        nc.sync.dma_start(fx1_lA[nb * 128:(nb + 1) * 128, :], fx[:])
                    else:
                        nc.sync.dma_start(fx1_lB[(nb - 5) * 128:(nb - 4) * 128, :],
                                          fx[:])
                    er = cp.tile([128, 8], bf16, tag=f"er1c{nb}", name=f"er1c{nb}")
                    nc.vector.tensor_copy(er[:], psLR[:, 8:16])
                    er1_sb.append(er)
                    if nb == 4:
                        nc.gpsimd.collective_compute(
                            "AllGather", mybir.AluOpType.bypass,
                            replica_groups=RG,
                            ins=[fx1_lA[:]], outs=[fx1_full[0:HFULL, :]])
                nc.gpsimd.collective_compute(
                    "AllGather", mybir.AluOpType.bypass,
                    replica_groups=RG,
                    ins=[fx1_lB[:]], outs=[fx1_full[HFULL:FULL, :]])
            hw_pool.__exit__(None, None, None)

            # ---- layer-1 edge processing: pass A then pass B ----
            # pools are shared across the two passes so no pool-boundary
            # barrier blocks pass B's table prefetch during pass A / the AG
            def l1_pass(half, ks, offs, srcI_t, dof_d, st_d, tab,
                        ppC, ppT, ppD, ep, pre=None):
                    for ch in range(NCHUNK):
                        k = ks[ch]
                        o = offs[ch]
                        if pre is None:
                            sdt = ep.tile([128, KT * 128], fp8, tag="sdt")
                            nc.sync.dma_start(sdt[:, 0:k * 128],
                                              dof_d[:, o * 128:(o + k) * 128])
                            stt = ep.tile([128, KT * 128], fp8, tag="stt")
                            nc.sync.dma_start(stt[:, 0:k * 128],
                                              st_d[:, o * 128:(o + k) * 128])
                            so = 0
                        else:
                            sdt, stt = pre
                            so = o
                        erp = ppC.tile([128, KT * 8], f32, tag="erp", bufs=1)
                        for jj in range(k):
                            nc.tensor.matmul(erp[:, jj * 8:(jj + 1) * 8],
                                             sdt[:, (so + jj) * 128:
                                                 (so + jj + 1) * 128],
                                             er1_sb[ch][:],
                                             start=(jj == 0), stop=(jj == k - 1),
                                             skip_group_check=True)
                        pa = ppC.tile([128, 512], f32, tag="agg")
                        pss = ppC.tile([128, 8], f32, tag="ss")
                        g = ep.tile([128, KT, 528], u16, tag="g", bufs=4)
                        for q in range(4):
                            a = k * q // 4
                            b = k * (q + 1) // 4
                            if b <= a:
                                continue
                            _gather_fn()(
                                nc.gpsimd,
                                g[:, a:b, :], tab[:, 0:528],
                                srcI_t[:, (o + a) * 8:(o + b) * 8],
                                num_idxs=(b - a) * 128,
                                num_idxs_reg=(b - a) * 128,
                                elem_size=528, elem_step=FXW,
                                single_packet=True, queue_num=q)
                        for j0 in range(0, k, SLAB_BATCH):
                            nb_ = min(SLAB_BATCH, k - j0)
                            jsl = slice(j0, j0 + nb_)
                            lr = ep.tile([128, SLAB_BATCH, 8], f32, tag="lr",
                                         bufs=3)
                            nc.vector.tensor_tensor(
                                lr[:, 0:nb_, :],
                                g[:, jsl, 512:528].bitcast(f32),
                                erp[:, j0 * 8:(j0 + nb_) * 8]
                                    .rearrange("p (b n) -> p b n", n=8),
                                Alu.add)
                            nc.vector.scalar_tensor_tensor(
                                lr[:, 0:nb_, :], lr[:, 0:nb_, :], NEG,
                                lr[:, 0:nb_, :], Alu.mult, Alu.max)
                            exb = ep.tile([128, SLAB_BATCH, 8], bf16, tag="exb",
                                          bufs=3)
                            nc.scalar.activation(exb[:, 0:nb_, :],
                                                 lr[:, 0:nb_, :], Act.Exp)
                            xt = ep.tile([128, SLAB_BATCH, 512], bf16, tag="xt",
                                         bufs=3)
                            nc.vector.tensor_tensor(
                                xt[:, 0:nb_, :]
                                    .rearrange("p b (d h) -> p b d h", h=H0),
                                g[:, jsl, 0:512].bitcast(bf16)
                                    .rearrange("p b (d h) -> p b d h", h=H0),
                                exb[:, 0:nb_, :].unsqueeze(2)
                                    .broadcast_to([128, nb_, HID, H0]),
                                Alu.mult)
                            for j in range(nb_):
                                jj = j0 + j
                                stj = stt[:, (so + jj) * 128:
                                           (so + jj + 1) * 128]
                                nc.tensor.matmul(pa[:], stj, xt[:, j, :],
                                                 start=(jj == 0),
                                                 stop=(jj == k - 1))
                                nc.tensor.matmul(pss[:], stj, exb[:, j, :],
                                                 start=(jj == 0),
                                                 stop=(jj == k - 1))
                        if half == "A":
                            a1 = cp.tile([128, 512], f32, tag=f"a1_{ch}",
                                         name=f"a1_{ch}")
                            nc.vector.tensor_copy(a1[:], pa[:])
                            s1 = cp.tile([128, 8], f32, tag=f"s1_{ch}",
                                         name=f"s1_{ch}")
                            nc.vector.tensor_copy(s1[:], pss[:])
                            acc1_sb.append(a1)
                            accS1_sb.append(s1)
                            continue
                        # ---- pass B tail: combine + normalize + ELU + layer-2
                        # feature row ----
                        sden = ep.tile([128, 8], f32, tag="sden")
                        nc.vector.tensor_tensor(sden[:], pss[:],
                                                accS1_sb[ch][:], Alu.add)
                        nc.vector.tensor_scalar(sden[:], sden[:], 1e-30, None,
                                                Alu.max)
                        rec = ep.tile([128, 8], f32, tag="rec")
                        nc.vector.reciprocal(rec[:], sden[:])
                        xs = ep.tile([128, 512], f32, tag="xs", bufs=1)
                        nc.vector.tensor_tensor(xs[:], pa[:], acc1_sb[ch][:],
                                                Alu.add)
                        x1 = ep.tile([128, 512], f32, tag="x1", bufs=1)
                        nc.vector.tensor_tensor(
                            x1[:].rearrange("p (d h) -> p d h", h=H0),
                            xs[:].rearrange("p (d h) -> p d h", h=H0),
                            rec[:].unsqueeze(1).broadcast_to([128, HID, H0]),
                            Alu.mult)
                        # ELU: relu(v) - 1 + exp(min(v, 0)); min via relu(-v)
                        tng = ep.tile([128, 512], f32, tag="tng", bufs=1)
                        nc.scalar.activation(tng[:], x1[:], Act.Relu, scale=-1.0)
                        texp = ep.tile([128, 512], f32, tag="texp", bufs=1)
                        nc.scalar.activation(texp[:], tng[:], Act.Exp, scale=-1.0)
                        trelu = ep.tile([128, 512], f32, tag="trelu", bufs=1)
                        nc.scalar.activation(trelu[:], x1[:], Act.Relu)
                        xe = ep.tile([128, 512], f32, tag="xe", bufs=1)
                        nc.vector.scalar_tensor_tensor(xe[:], texp[:], -1.0,
                                                       trelu[:], Alu.add, Alu.add)
                        xTc = []
                        for s in range(K1):
                            tp = ppT.tile([128, 128], f32, tag="tp", bufs=1)
                            nc.tensor.transpose(tp[:],
                                                xe[:, s * 128:(s + 1) * 128],
                                                id_t[:])
                            xc = wp.tile([128, 128], bf16, tag=f"xTc{s}")
                            nc.vector.tensor_copy(xc[:], tp[:])
                            xTc.append(xc)
                        psD = ppD.tile([128, 66], f32, tag="feat2", bufs=2)
                        for s in range(K1):
                            nc.tensor.matmul(psD[:], xTc[s][:], W2F_b[:, s, :],
                                             start=(s == 0), stop=(s == K1 - 1))
                        fx2 = wp.tile([128, 128], u16, tag="fx2")
                        nc.vector.tensor_copy(fx2[:, 0:64].bitcast(bf16),
                                              psD[:, 0:64])
                        nc.vector.tensor_copy(fx2[:, 64:66].bitcast(f32),
                                              psD[:, 64:65])
                        er2 = cp.tile([128, 1], bf16, tag=f"er2c{ch}",
                                      name=f"er2c{ch}")
                        nc.vector.tensor_copy(er2[:], psD[:, 65:66])
                        er2_sb.append(er2)
                        if ch < 5:
                            nc.sync.dma_start(
                                fx2_lA[ch * 128:(ch + 1) * 128, :], fx2[:])
                        else:
                            nc.sync.dma_start(
                                fx2_lB[(ch - 5) * 128:(ch - 4) * 128, :], fx2[:])
                        if ch == 4:
                            nc.gpsimd.collective_compute(
                                "AllGather", mybir.AluOpType.bypass,
                                replica_groups=RG,
                                ins=[fx2_lA[:]], outs=[fx2_full[0:HFULL, :]])

            with (
                tc.tile_pool(name="ppC", bufs=2, space="PSUM") as ppC,
                tc.tile_pool(name="ppT", bufs=2, space="PSUM") as ppT,
                tc.tile_pool(name="ppD", bufs=2, space="PSUM") as ppD,
                tc.tile_pool(name="edge", bufs=2) as ep,
            ):
                l1_pass("A", KA, offA, srcIA_t, dofA_d[:], stA_d[:],
                        fx1_full[0:HFULL, :], ppC, ppT, ppD, ep, pre=preA)
                l1_pass("B", KB, offB, srcIB_t, dofB_d[:], stB_d[:],
                        fx1_full[HFULL:FULL, :], ppC, ppT, ppD, ep)

            nc.gpsimd.collective_compute(
                "AllGather", mybir.AluOpType.bypass,
                replica_groups=RG,
                ins=[fx2_lB[:]], outs=[fx2_full[HFULL:FULL, :]])

            # ---- layer-2 edge processing: pass A then pass B ----
            def l2_pass(half, ks, offs, srcI_t, dof_d, st_d, tab, ppE, e2,
                        pre=None):
                    for ch in range(NCHUNK):
                        k = ks[ch]
                        o = offs[ch]
                        if pre is None:
                            sdt = e2.tile([128, KT * 128], fp8, tag="sdt2")
                            nc.sync.dma_start(sdt[:, 0:k * 128],
                                              dof_d[:, o * 128:(o + k) * 128])
                            stt = e2.tile([128, KT * 128], fp8, tag="stt2")
                            nc.sync.dma_start(stt[:, 0:k * 128],
                                              st_d[:, o * 128:(o + k) * 128])
                            so = 0
                        else:
                            sdt, stt = pre
                            so = o
                        erp2 = ppE.tile([128, KT], f32, tag="erp2", bufs=1)
                        for jj in range(k):
                            nc.tensor.matmul(erp2[:, jj:jj + 1],
                                             sdt[:, (so + jj) * 128:
                                                 (so + jj + 1) * 128],
                                             er2_sb[ch][:],
                                             start=(jj == 0), stop=(jj == k - 1),
                                             skip_group_check=True)
                        pa2 = ppE.tile([128, 64], f32, tag="agg2")
                        pss2 = ppE.tile([128, 1], f32, tag="ss2")
                        g2 = e2.tile([128, KT, 128], u16, tag="g2", bufs=6)
                        for q in range(4):
                            a = k * q // 4
                            b = k * (q + 1) // 4
                            if b <= a:
                                continue
                            nc.gpsimd.dma_gather(
                                g2[:, a:b, :], tab,
                                srcI_t[:, (o + a) * 8:(o + b) * 8],
                                num_idxs=(b - a) * 128,
                                num_idxs_reg=(b - a) * 128,
                                elem_size=128, single_packet=True,
                                queue_num=q)
                        for j0 in range(0, k, SLAB_BATCH):
                            nb_ = min(SLAB_BATCH, k - j0)
                            jsl = slice(j0, j0 + nb_)
                            lr2 = e2.tile([128, SLAB_BATCH, 1], f32, tag="lr2",
                                          bufs=3)
                            nc.vector.tensor_tensor(
                                lr2[:, 0:nb_, :],
                                g2[:, jsl, 64:66].bitcast(f32),
                                erp2[:, j0:j0 + nb_]
                                    .rearrange("p (b n) -> p b n", n=1),
                                Alu.add)
                            nc.vector.scalar_tensor_tensor(
                                lr2[:, 0:nb_, :], lr2[:, 0:nb_, :], NEG,
                                lr2[:, 0:nb_, :], Alu.mult, Alu.max)
                            exb2 = e2.tile([128, SLAB_BATCH, 1], bf16,
                                           tag="exb2", bufs=3)
                            nc.scalar.activation(exb2[:, 0:nb_, :],
                                                 lr2[:, 0:nb_, :], Act.Exp)
                            xt2 = e2.tile([128, SLAB_BATCH, 64], bf16,
                                          tag="xt2", bufs=3)
                            nc.vector.tensor_tensor(
                                xt2[:, 0:nb_, :],
                                g2[:, jsl, 0:64].bitcast(bf16),
                                exb2[:, 0:nb_, :]
                                    .broadcast_to([128, nb_, 64]),
                                Alu.mult)
                            for j in range(nb_):
                                jj = j0 + j
                                stj = stt[:, (so + jj) * 128:
                                           (so + jj + 1) * 128]
                                nc.tensor.matmul(pa2[:], stj, xt2[:, j, :],
                                                 start=(jj == 0),
                                                 stop=(jj == k - 1))
                                nc.tensor.matmul(pss2[:], stj, exb2[:, j, :],
                                                 start=(jj == 0),
                                                 stop=(jj == k - 1))
                        if half == "A":
                            a2 = cp.tile([128, 64], f32, tag=f"a2_{ch}",
                                         name=f"a2_{ch}")
                            nc.vector.tensor_copy(a2[:], pa2[:])
                            s2 = cp.tile([128, 1], f32, tag=f"s2_{ch}",
                                         name=f"s2_{ch}")
                            nc.vector.tensor_copy(s2[:], pss2[:])
                            acc2_sb.append(a2)
                            accS2_sb.append(s2)
                            continue
                        sden2 = e2.tile([128, 1], f32, tag="sden2")
                        nc.vector.tensor_tensor(sden2[:], pss2[:],
                                                accS2_sb[ch][:], Alu.add)
                        nc.vector.tensor_scalar(sden2[:], sden2[:], 1e-30,
                                                None, Alu.max)
                        rec2 = e2.tile([128, 1], f32, tag="rec2")
                        nc.vector.reciprocal(rec2[:], sden2[:])
                        xo = e2.tile([128, 64], f32, tag="xo")
                        nc.vector.tensor_tensor(xo[:], pa2[:], acc2_sb[ch][:],
                                                Alu.add)
                        outf = e2.tile([128, 64], f32, tag="outf")
                        nc.vector.tensor_tensor(
                            outf[:], xo[:],
                            rec2[:].broadcast_to([128, 64]), Alu.mult)
                        nc.sync.dma_start(y_d[ch * 128:(ch + 1) * 128, :],
                                          outf[:])

            with (
                tc.tile_pool(name="ppE", bufs=2, space="PSUM") as ppE,
                tc.tile_pool(name="edge2", bufs=2) as e2,
            ):
                l2_pass("A", KA, offA, srcIA_t, dofA_d[:], stA_d[:],
                        fx2_full[0:HFULL, :], ppE, e2, pre=preA)
                l2_pass("B", KB, offB, srcIB_t, dofB_d[:], stB_d[:],
                        fx2_full[HFULL:FULL, :], ppE, e2)

    nc.compile()
    return nc


def _wrap_idx(a):
    """flat int array -> [128, n//16] int16 dma_gather index layout."""
    w = a.reshape(-1, 16).T.astype(np.int16)
    return np.tile(w, (8, 1))


def _prep_inputs(h, src, dst, W1, attn_l1, attn_r1, W2, attn_l2, attn_r2):
    src = np.asarray(src)
    dst = np.asarray(dst)
    h = np.asarray(h, dtype=np.float32)
    W1 = np.asarray(W1, dtype=np.float32)
    W2 = np.asarray(W2, dtype=np.float32)
    al1 = np.asarray(attn_l1, np.float32)
    ar1 = np.asarray(attn_r1, np.float32)
    al2 = np.asarray(attn_l2, np.float32).reshape(-1)
    ar2 = np.asarray(attn_r2, np.float32).reshape(-1)

    # head-interleaved feature column permutation: new col d*8+h <- old h*64+d
    jj = np.arange(IN_DIM)
    pw1 = (jj % H0) * HID + (jj // H0)     # old col for new col j
    W1X = W1[:, pw1]
    Al1f = np.zeros((IN_DIM, H0), np.float32)
    Ar1f = np.zeros((IN_DIM, H0), np.float32)
    Al1f[jj, jj % H0] = al1[jj % H0, jj // H0]
    Ar1f[jj, jj % H0] = ar1[jj % H0, jj // H0]
    W1F = np.concatenate([W1X, W1X @ Al1f, W1X @ Ar1f], axis=1)
    # rows of W2 permuted to match interleaved layer-1 output columns
    W2X = W2[pw1, :]
    W2F = np.concatenate([W2X, (W2X @ al2)[:, None], (W2X @ ar2)[:, None]],
                         axis=1)

    # half-split global row numbering
    nn = np.arange(N_NODES)
    cc = nn // NPER
    ll = nn % NPER
    gid = np.where(ll < HL, cc * HL + ll, HFULL + cc * HL + (ll - HL))

    core_of = dst // NPER
    dloc_all = dst % NPER

    edA, edB = [], []          # per core: lists of (src_gid, dloc) per chunk
    nAs = np.zeros((NCORES, NCHUNK), int)
    nBs = np.zeros((NCORES, NCHUNK), int)
    for c in range(NCORES):
        ids = np.nonzero(core_of == c)[0]
        dl = dloc_all[ids]
        ch = dl // 128
        gsrc = gid[src[ids]]
        isA = gsrc < HFULL
        rowsA, rowsB = [], []
        for k in range(NCHUNK):
            mA = (ch == k) & isA
            mB = (ch == k) & ~isA
            iA = ids[mA][np.argsort(gsrc[mA], kind="stable")]
            iB = ids[mB][np.argsort(gsrc[mB], kind="stable")]
            rowsA.append((gid[src[iA]], dloc_all[iA] - 128 * k))
            rowsB.append((gid[src[iB]] - HFULL, dloc_all[iB] - 128 * k))
            nAs[c, k] = len(iA)
            nBs[c, k] = len(iB)
        edA.append(rowsA)
        edB.append(rowsB)

    KA = tuple(int(np.ceil(nAs[:, k].max() / 128)) for k in range(NCHUNK))
    KB = tuple(int(np.ceil(nBs[:, k].max() / 128)) for k in range(NCHUNK))

    def build_tables(rows_per_core_chunk, KS):
        """-> (srcI wrapped, dof one-hot fp8, st one-hot fp8) per core."""
        S = sum(KS)
        rng128 = np.arange(128)
        out = []
        for c in range(NCORES):
            src_g = np.zeros(S * 128, np.int64)
            doff = np.full(S * 128, 200, np.int64)
            pos = 0
            for k in range(NCHUNK):
                gs, dl = rows_per_core_chunk[c][k]
                n = len(gs)
                src_g[pos:pos + n] = gs
                doff[pos:pos + n] = dl
                pos += KS[k] * 128
            doh = (doff[None, :] == rng128[:, None]).astype(
                ml_dtypes.float8_e4m3)
            D = doff.reshape(-1, 128)
            soh = np.ascontiguousarray(
                (D[:, :, None] == rng128[None, None, :])
                .transpose(1, 0, 2).reshape(128, S * 128)).astype(
                    ml_dtypes.float8_e4m3)
            out.append((_wrap_idx(src_g), doh, soh))
        return out

    tabA = build_tables(edA, KA)
    tabB = build_tables(edB, KB)

    ident = np.eye(128, dtype=np.float32)
    in_maps = []
    for c in range(NCORES):
        hc = np.zeros((IN_DIM, LOC), ml_dtypes.bfloat16)
        hc[:, :NPER] = h[c * NPER:(c + 1) * NPER].T.astype(ml_dtypes.bfloat16)
        in_maps.append({
            "hT": hc,
            "W1F": W1F.astype(ml_dtypes.bfloat16),
            "W2F": W2F.astype(ml_dtypes.bfloat16),
            "srcIA": tabA[c][0], "srcIB": tabB[c][0],
            "dofA": tabA[c][1], "dofB": tabB[c][1],
            "stA": tabA[c][2], "stB": tabB[c][2],
            "ident": ident,
        })
    return KA, KB, in_maps


def kernel(h, src, dst, W1, attn_l1, attn_r1, W2, attn_l2, attn_r2,
           _trace=False):
    from concourse.bass_utils import run_bass_kernel_spmd

    KA, KB, in_maps = _prep_inputs(h, src, dst, W1, attn_l1, attn_r1,
                                   W2, attn_l2, attn_r2)
    key = (KA, KB)
    if key not in _cache:
        _cache[key] = _build(KA, KB)
    nc = _cache[key]

    if _trace:
        _install_trace_hook()
    res = run_bass_kernel_spmd(nc, in_maps, list(range(NCORES)), trace=_trace)
    out = np.concatenate([res.results[c]["y"][:NPER] for c in range(NCORES)],
                         axis=0)
    if _trace:
        return out, res
    return out


def _install_trace_hook():
    import sys, types
    if "antenv.axon_hooks" in sys.modules:
        return
    try:
        import antenv
        from trn_agent_boot.trn_boot import _ntff_profile_via_ctypes
    except ImportError:
        return
    mod = types.ModuleType("antenv.axon_hooks")
    state = {"hook": None}
    mod.set_axon_ntff_profile_hook = lambda hk: state.__setitem__("hook", hk)
    mod.get_axon_ntff_profile_hook = lambda: state["hook"]
    sys.modules["antenv.axon_hooks"] = mod
    antenv.axon_hooks = mod
    try:
        mod.set_axon_ntff_profile_hook(
            _ntff_profile_via_ctypes("/opt/axon/libaxon_pjrt.so"))
    except Exception:
        pass
